# revision 6
# baseline (speedup 1.0000x reference)
"""GaborConv2d Trainium2 kernel.

Strategy
--------
Host: generate the tiny [64,3,7,7] Gabor weights, pad the input, and build a
96-plane two-row im2col stack per image: output rows are processed in PAIRS
(r=2p, r=2p+1); plane s=(dy*12 + c*4 + u) of pair p holds pad[c, 2p+dy, u+x]
for dy in 0..7, c in 0..2, u in 0..3.

Device (per core, 2 images batch-sharded): each matmul computes BOTH rows of
a pair at once: stationary [96, 128] maps plane (dy,c,u) -> output (rr*64+o)
with weight W[o,c,dy-rr,4t+u]; 2 accumulating supertaps t (kj=4t+u) with
moving [96, 512] read at free offset 4t. So one pair = 2 matmuls (vs 28
K=3 naive taps). PSUM [128,512] = one bank per pair; sweeps of 8 pairs
iterate t-outer/bank-inner so each stationary streams across 8 banks.
Vector/Scalar/GpSimd rotate evicting banks as f32->bf16 into a 32-pair
staging tile stored as one 4MB DMA (32KB/partition lines). Output DRAM
layout is [img, rr, o, pair, x]; host reinterleaves rows at the end.
"""

import math

import ml_dtypes
import numpy as np

import concourse.bass as bass
import concourse.mybir as mybir
import concourse.tile as tile
from concourse import bacc
from concourse.bass_utils import run_bass_kernel_spmd

F32 = mybir.dt.float32
BF16 = mybir.dt.bfloat16
BF16NP = ml_dtypes.bfloat16

N_CORES = 8
B, C, H, W = 16, 3, 512, 512
O, K, PAD = 64, 7, 3
IPC = B // N_CORES          # images per core
NP = 96                     # planes: dy(8) * c(3) * u(4)
XW = 520                    # stored plane width
NPAIR = H // 2              # 256 row pairs per image
PB = 32                     # pairs per block (4 sweeps of 8)
NT = 2                      # supertaps, kj = 4t+u
HPAD = H + 2 * PAD          # 518 padded rows
WPAD = 524                  # padded width (3 + 512 + 3, +u slack, even)
DELTA = 0.001


def _gabor_weights(freq, theta, sigma, psi):
    x0 = math.ceil(K / 2)
    lin = np.linspace(-x0 + 1, x0, K, dtype=np.float32)
    y = np.broadcast_to(lin[:, None], (K, K))
    x = np.broadcast_to(lin[None, :], (K, K))
    th = theta[:, :, None, None].astype(np.float32)
    fr = freq[:, :, None, None].astype(np.float32)
    sg = sigma[:, :, None, None].astype(np.float32)
    ps = psi[:, :, None, None].astype(np.float32)
    rotx = x * np.cos(th) + y * np.sin(th)
    roty = -x * np.sin(th) + y * np.cos(th)
    g = np.exp(-0.5 * ((rotx**2 + roty**2) / (sg + DELTA) ** 2))
    g = g * np.cos(fr * rotx + ps)
    g = g / (2 * np.pi * sg**2)
    return g.astype(np.float32)  # [O, C, K, K]


def _build_nc():
    nc = bacc.Bacc(None, target_bir_lowering=False)
    # (img, plane, paircol, x)
    xs = nc.dram_tensor("xstack", [IPC, NP, NPAIR, XW], BF16,
                        kind="ExternalInput")
    wb = nc.dram_tensor("wbig", [NP, NT * 128], BF16, kind="ExternalInput")
    # (img, rr, o, pair, x)
    y = nc.dram_tensor("y", [IPC, 2, O, NPAIR, W], BF16, kind="ExternalOutput")

    with tile.TileContext(nc) as tc:
        with (
            tc.tile_pool(name="wpool", bufs=1) as wpool,
            tc.tile_pool(name="ipool", bufs=3) as ipool,
            tc.tile_pool(name="spool", bufs=2) as spool,
            tc.tile_pool(name="ppool", bufs=8, space="PSUM") as ppool,
        ):
            wt = wpool.tile([NP, NT * 128], BF16)
            nc.sync.dma_start(out=wt, in_=wb[:])

            for img in range(IPC):
                for blk in range(NPAIR // PB):
                    it = ipool.tile([NP, PB * XW], BF16, tag="img")
                    nc.scalar.dma_start(
                        out=it,
                        in_=bass.AP(
                            xs,
                            img * (NP * NPAIR * XW) + blk * PB * XW,
                            [[NPAIR * XW, NP], [1, PB * XW]],
                        ),
                    )
                    stg = spool.tile([128, PB * W], BF16, tag="stg")
                    for sweep in range(PB // 8):
                        pss = [
                            ppool.tile([128, W], F32, tag="ps", name=f"ps{b}")
                            for b in range(8)
                        ]
                        for t in range(NT):
                            for b in range(8):
                                pl = sweep * 8 + b
                                nc.tensor.matmul(
                                    pss[b][:, :],
                                    wt[:, t * 128 : (t + 1) * 128],
                                    it[:, pl * XW + 4 * t : pl * XW + 4 * t + W],
                                    start=(t == 0),
                                    stop=(t == NT - 1),
                                )
                        for b in range(8):
                            pl = sweep * 8 + b
                            sl = stg[:, pl * W : (pl + 1) * W]
                            if b % 2 == 0:
                                nc.vector.tensor_copy(sl, pss[b][:, :])
                            else:
                                nc.scalar.copy(sl, pss[b][:, :])
                    nc.sync.dma_start(
                        out=bass.AP(
                            y,
                            img * (2 * O * NPAIR * W) + blk * PB * W,
                            [[NPAIR * W, 2 * O], [1, PB * W]],
                        ),
                        in_=stg,
                    )
    nc.finalize()
    return nc


def _prepare_inputs(input_tensor, freq, theta, sigma, psi):
    g = _gabor_weights(freq, theta, sigma, psi)  # [O, C, K, K] f32
    # wbig[dy*12+c*4+u, t*128 + rr*64 + o] = g[o, c, dy-rr, 4t+u]
    wmat = np.zeros((NP, NT * 128), np.float32)
    for t in range(NT):
        for dy in range(8):
            for c in range(C):
                for u in range(4):
                    kj = 4 * t + u
                    if kj >= K:
                        continue
                    s = dy * 12 + c * 4 + u
                    for rr in range(2):
                        ki = dy - rr
                        if not (0 <= ki < K):
                            continue
                        col = t * 128 + rr * 64
                        wmat[s, col : col + O] = g[:, c, ki, kj]
    wbig = wmat.astype(BF16NP)

    xb = input_tensor.astype(BF16NP)
    pad = np.zeros((B, C, HPAD, WPAD), BF16NP)
    pad[:, :, PAD : PAD + H, PAD : PAD + W] = xb
    # xstack[img, dy*12+c*4+u, p, x] = pad[img, c, 2p+dy, u+x]
    in_maps = []
    for core in range(N_CORES):
        imgs = pad[core * IPC : (core + 1) * IPC]
        xstack = np.empty((IPC, NP, NPAIR, XW), BF16NP)
        for dy in range(8):
            for c in range(C):
                for u in range(4):
                    s = dy * 12 + c * 4 + u
                    xstack[:, s] = imgs[:, c, dy : dy + 2 * NPAIR - 1 : 2, u : u + XW]
        in_maps.append({"xstack": xstack, "wbig": wbig})
    return in_maps


_NC_CACHE = None


def kernel(input_tensor, freq, theta, sigma, psi):
    global _NC_CACHE
    input_tensor = np.asarray(input_tensor, dtype=np.float32)
    in_maps = _prepare_inputs(
        input_tensor,
        np.asarray(freq), np.asarray(theta), np.asarray(sigma), np.asarray(psi),
    )
    if _NC_CACHE is None:
        _NC_CACHE = _build_nc()
    res = run_bass_kernel_spmd(_NC_CACHE, in_maps, core_ids=list(range(N_CORES)))
    out = np.empty((B, O, H, W), np.float32)
    for core in range(N_CORES):
        ydev = res.results[core]["y"]  # [IPC, 2, O, NPAIR, W] bf16
        for i in range(IPC):
            img = core * IPC + i
            out[img, :, 0::2, :] = ydev[i, 0]
            out[img, :, 1::2, :] = ydev[i, 1]
    return out


# revision 7
# speedup vs baseline: 1.4758x; 1.4758x over previous
"""GaborConv2d Trainium2 kernel.

Strategy
--------
Host: generate the tiny [64,3,7,7] Gabor weights, pad the input, and build a
48-plane two-row im2col stack per image: output rows are processed in PAIRS
(r=2p, r=2p+1); plane s=(dy*6 + c*2 + u) of pair p holds pad[c, 2p+dy, u+x].
Even pairs go to partition group 0 (partitions 0..47), odd pairs to group 1
(partitions 64..111).

Device (per core, 2 images batch-sharded): each matmul computes BOTH rows of
a pair at once: stationary [48, 128] maps plane (dy,c,u) -> output (rr*64+o)
with weight W[o,c,dy-rr,2t+u]; 4 accumulating supertaps t (kj=2t+u) with
moving [48, 512] read at free offset 2t. K=48 <= 64 keeps the PE in its
0.25ns/col zone (K>64 halves throughput). PSUM [128,512] = one bank per
pair; sub-sweeps of 4 pairs iterate t-outer/bank-inner; while banks 4-7
compute, banks 0-3 are evicted (f32->bf16) by VectorE/ScalarE into a
16-pair staging tile - eviction (2x606ns) hides fully under 16 matmuls.
Stores are one 1MB DMA per 8 pairs (8KB/partition lines). Output DRAM
layout is [img, rr, o, pair, x]; host reinterleaves rows at the end.
"""

import math

import ml_dtypes
import numpy as np

import concourse.bass as bass
import concourse.mybir as mybir
import concourse.tile as tile
from concourse import bacc
from concourse.bass_utils import run_bass_kernel_spmd

F32 = mybir.dt.float32
BF16 = mybir.dt.bfloat16
BF16NP = ml_dtypes.bfloat16

N_CORES = 8
B, C, H, W = 16, 3, 512, 512
O, K, PAD = 64, 7, 3
IPC = B // N_CORES          # images per core
NP = 48                     # planes: dy(8) * c(3) * u(2)
XW = 520                    # stored plane width
NPAIR = H // 2              # 256 row pairs per image
PB = 16                     # pairs per block
NT = 4                      # supertaps, kj = 2t+u
HPAD = H + 2 * PAD          # 518 padded rows
WPAD = 524                  # padded width (3 + 512 + 3, +u slack, even)
DELTA = 0.001


def _gabor_weights(freq, theta, sigma, psi):
    x0 = math.ceil(K / 2)
    lin = np.linspace(-x0 + 1, x0, K, dtype=np.float32)
    y = np.broadcast_to(lin[:, None], (K, K))
    x = np.broadcast_to(lin[None, :], (K, K))
    th = theta[:, :, None, None].astype(np.float32)
    fr = freq[:, :, None, None].astype(np.float32)
    sg = sigma[:, :, None, None].astype(np.float32)
    ps = psi[:, :, None, None].astype(np.float32)
    rotx = x * np.cos(th) + y * np.sin(th)
    roty = -x * np.sin(th) + y * np.cos(th)
    g = np.exp(-0.5 * ((rotx**2 + roty**2) / (sg + DELTA) ** 2))
    g = g * np.cos(fr * rotx + ps)
    g = g / (2 * np.pi * sg**2)
    return g.astype(np.float32)  # [O, C, K, K]


def _build_nc():
    nc = bacc.Bacc(None, target_bir_lowering=False)
    # (img, parity, plane, paircol, x)
    xs = nc.dram_tensor("xstack", [IPC, 2, NP, NPAIR // 2, XW], BF16,
                        kind="ExternalInput")
    wb = nc.dram_tensor("wbig", [128, NT * 128], BF16, kind="ExternalInput")
    # (img, rr, o, pair, x)
    y = nc.dram_tensor("y", [IPC, 2, O, NPAIR, W], BF16, kind="ExternalOutput")

    PCB = PB // 2  # paircols per group per block

    with tile.TileContext(nc) as tc:
        with (
            tc.tile_pool(name="wpool", bufs=1) as wpool,
            tc.tile_pool(name="ipool", bufs=4) as ipool,
            tc.tile_pool(name="spool", bufs=3) as spool,
            tc.tile_pool(name="ppool", bufs=8, space="PSUM") as ppool,
        ):
            wt = wpool.tile([128, NT * 128], BF16)
            nc.sync.dma_start(out=wt, in_=wb[:])

            for img in range(IPC):
                for blk in range(NPAIR // PB):
                    it = ipool.tile([128, PCB * XW], BF16, tag="img")
                    for gg in range(2):
                        nc.scalar.dma_start(
                            out=it[64 * gg : 64 * gg + NP, :],
                            in_=bass.AP(
                                xs,
                                ((img * 2 + gg) * NP) * (NPAIR // 2) * XW
                                + blk * PCB * XW,
                                [[(NPAIR // 2) * XW, NP], [1, PCB * XW]],
                            ),
                        )
                    stg = spool.tile([128, PB * W], BF16, tag="stg")
                    for half in range(PB // 8):
                        for sub in range(2):  # 4-pair sub-sweeps
                            pss = [
                                ppool.tile([128, W], F32, tag="ps", name=f"ps{b}")
                                for b in range(4)
                            ]
                            for t in range(NT):
                                for b in range(4):
                                    pl = half * 8 + sub * 4 + b
                                    gg = pl % 2
                                    pcol = pl // 2
                                    nc.tensor.matmul(
                                        pss[b][:, :],
                                        wt[64 * gg : 64 * gg + NP,
                                           t * 128 : (t + 1) * 128],
                                        it[64 * gg : 64 * gg + NP,
                                           pcol * XW + 2 * t
                                           : pcol * XW + 2 * t + W],
                                        start=(t == 0),
                                        stop=(t == NT - 1),
                                    )
                            for b in range(4):
                                pl = half * 8 + sub * 4 + b
                                sl = stg[:, pl * W : (pl + 1) * W]
                                if b % 2 == 0:
                                    nc.vector.tensor_copy(sl, pss[b][:, :])
                                else:
                                    nc.scalar.copy(sl, pss[b][:, :])
                        nc.sync.dma_start(
                            out=bass.AP(
                                y,
                                img * (2 * O * NPAIR * W)
                                + (blk * PB + half * 8) * W,
                                [[NPAIR * W, 2 * O], [1, 8 * W]],
                            ),
                            in_=stg[:, half * 8 * W : (half * 8 + 8) * W],
                        )
    nc.finalize()
    return nc


def _prepare_inputs(input_tensor, freq, theta, sigma, psi):
    g = _gabor_weights(freq, theta, sigma, psi)  # [O, C, K, K] f32
    # wbig[64*half + dy*6+c*2+u, t*128 + rr*64 + o] = g[o, c, dy-rr, 2t+u]
    wmat = np.zeros((128, NT * 128), np.float32)
    for t in range(NT):
        for dy in range(8):
            for c in range(C):
                for u in range(2):
                    kj = 2 * t + u
                    if kj >= K:
                        continue
                    s = dy * 6 + c * 2 + u
                    for rr in range(2):
                        ki = dy - rr
                        if not (0 <= ki < K):
                            continue
                        col = t * 128 + rr * 64
                        wmat[s, col : col + O] = g[:, c, ki, kj]
                        wmat[64 + s, col : col + O] = g[:, c, ki, kj]
    wbig = wmat.astype(BF16NP)

    xb = input_tensor.astype(BF16NP)
    pad = np.zeros((B, C, HPAD, WPAD), BF16NP)
    pad[:, :, PAD : PAD + H, PAD : PAD + W] = xb
    # xstack[img, gg, dy*6+c*2+u, q, x] = pad[img, c, 2*(2q+gg)+dy, u+x]
    in_maps = []
    for core in range(N_CORES):
        imgs = pad[core * IPC : (core + 1) * IPC]
        xstack = np.empty((IPC, 2, NP, NPAIR // 2, XW), BF16NP)
        for gg in range(2):
            for dy in range(8):
                for c in range(C):
                    for u in range(2):
                        s = dy * 6 + c * 2 + u
                        r0 = 2 * gg + dy
                        xstack[:, gg, s] = imgs[
                            :, c, r0 : r0 + 4 * (NPAIR // 2) : 4, u : u + XW
                        ]
        in_maps.append({"xstack": xstack, "wbig": wbig})
    return in_maps


_NC_CACHE = None


def kernel(input_tensor, freq, theta, sigma, psi):
    global _NC_CACHE
    input_tensor = np.asarray(input_tensor, dtype=np.float32)
    in_maps = _prepare_inputs(
        input_tensor,
        np.asarray(freq), np.asarray(theta), np.asarray(sigma), np.asarray(psi),
    )
    if _NC_CACHE is None:
        _NC_CACHE = _build_nc()
    res = run_bass_kernel_spmd(_NC_CACHE, in_maps, core_ids=list(range(N_CORES)))
    out = np.empty((B, O, H, W), np.float32)
    for core in range(N_CORES):
        ydev = res.results[core]["y"]  # [IPC, 2, O, NPAIR, W] bf16
        for i in range(IPC):
            img = core * IPC + i
            out[img, :, 0::2, :] = ydev[i, 0]
            out[img, :, 1::2, :] = ydev[i, 1]
    return out
